# revision 3
# baseline (speedup 1.0000x reference)
"""DGCNN_Propagation Trainium2 Bass kernel, v3 (int8 transport).

Data-parallel over batch: 16 samples -> 8 NeuronCores, 2 samples/core.

Key design points (driven by axon-tunnel profiling):
  - ONE packed f32 dynamic input blob per core + ONE f32 weight blob
    (device-resident across calls, revalidated by byte-compare) + ONE f32
    output blob: each extra jax array costs ~75ms of RPC latency per call,
    and non-f32 IO dtypes add fixed per-exec penalties (int16 outputs:
    ~130ms each!).  All f16/fp8 payloads are bitcast views inside f32 blobs.
  - f / f_q ship as per-(sample,channel) scaled int8 and are dequantized
    to f16 on device before the matmuls (weights stay f16).  ~0.9% RMS
    quantization noise vs fp8's 3.6%, at the same 1 byte/value.
  - kNN scores computed EXACTLY in fp32 on the TensorE (4-row matmul:
    2q.k - |k|^2; per-query constant q^2 dropped as it can't change
    ranking).  No coarse/refine split, no dma_gather, no kr tables.
  - Conv folding: W @ [gather(f)-xq; xq] == gather(Wa @ f) + (Wb-Wa) @ xq,
    so matmuls run on ungathered data and the gather (gpsimd ap_gather)
    runs per conv-output channel plane.
  - GroupNorm via per-partition accumulators + tiny selector matmuls;
    max-over-k pulled before the monotone affine; affine+LeakyReLU fused
    into one ACT Prelu op.  Activations f16.
"""

import numpy as np

import concourse.bass as bass
import concourse.bacc as bacc
import concourse.mybir as mybir
from concourse import bass2jax
from concourse.tile import TileContext

dt = mybir.dt
AF = mybir.ActivationFunctionType
ALU = mybir.AluOpType

P = 128
B, C, GS, GD, K = 16, 384, 4096, 1024, 4
BC = 2              # samples per core
NCORE = 8
NT = GD // P        # 8 query tiles
EPS = 1e-5
ALPHA = 0.2

f16 = dt.float16
f32 = dt.float32
i8 = dt.int8
i16 = dt.int16

# ---------------- blob layouts (f32 words, per core) ----------------


def _mk_layout(specs):
    off, sec = 0, {}
    for name, words in specs:
        sec[name] = (off, words)
        off += (words + 127) & ~127
    return sec, (off + 511) & ~511


_SEC, NW = _mk_layout([
    ("fs", BC * C * GS // 4),     # i8 [BC, 384, 4096], per-channel scaled
    ("fq", BC * C * GD // 4),     # i8 [BC, 384, 1024], per-channel scaled
    ("fssc", BC * C),             # f32 [BC, 3, 128] dequant scales (ko p)
    ("fqsc", BC * C),             # f32 [BC, 3, 128]
    ("lq", BC * 4 * GD),          # f32 [BC, 4, 1024]  rows: q0,q1,q2,1
    ("rk1", BC * 4 * GS),         # f32 [BC, 4, 4096]  rows: 2k0,2k1,2k2,-k2
    ("rk2", BC * 4 * GD),         # f32 [BC, 4, 1024]
])

_WSEC, WW = _mk_layout([
    ("w1a", C * 512 // 2),        # f16 [384, 512]
    ("w1d", C * 512 // 2),
    ("w2a", 512 * C // 2),        # f16 [512, 384]
    ("w2d", 512 * C // 2),
    ("g1t", P * 4),               # f32 [128, 4]
    ("b1t", P * 4),
    ("g2t", P * 3),
    ("b2t", P * 3),
    ("sel1", P * 4 * 4),
    ("sel1t", 4 * 4 * P),
    ("sel2", P * 3 * 4),
    ("sel2t", 4 * 3 * P),
])

OW = BC * C * GD // 2             # output blob words per core (f16 payload)


def _build():
    nc = bacc.Bacc("TRN2", target_bir_lowering=False, debug=False,
                   num_devices=NCORE)

    blob_d = nc.dram_tensor("blob", [1, NW], f32, kind="ExternalInput")
    wblob_d = nc.dram_tensor("wblob", [1, WW], f32, kind="ExternalInput")
    out_d = nc.dram_tensor("out", [1, OW], f32, kind="ExternalOutput")

    def bview(name):
        o, w = _SEC[name]
        return blob_d[0, o:o + w]

    def wview(name):
        o, w = _WSEC[name]
        return wblob_d[0, o:o + w]

    with TileContext(nc) as tc:
        with (
            tc.tile_pool(name="const", bufs=1) as cp,
            tc.tile_pool(name="big", bufs=1) as bp,
            tc.tile_pool(name="one", bufs=1) as op,
            tc.tile_pool(name="ta", bufs=2) as ta,    # ndt / u1c / u2c
            tc.tile_pool(name="tb", bufs=2) as tb,    # ug1c / ug2c / fp8 staging
            tc.tile_pool(name="sm", bufs=2) as sp,
            tc.tile_pool(name="pnd", bufs=2, space="PSUM") as pnd,
            tc.tile_pool(name="pcv", bufs=2, space="PSUM") as pcv,
            tc.tile_pool(name="pst", bufs=2, space="PSUM") as pst,
        ):
            # ---- constants (shared by both samples) ----
            w1a = cp.tile([P, 3, 512], f16)
            nc.sync.dma_start(w1a, wview("w1a").bitcast(f16).rearrange(
                "(ko p m) -> p ko m", p=P, m=512))
            w1d = cp.tile([P, 3, 512], f16)
            nc.sync.dma_start(w1d, wview("w1d").bitcast(f16).rearrange(
                "(ko p m) -> p ko m", p=P, m=512))
            w2a = cp.tile([P, 4, C], f16)
            nc.sync.dma_start(w2a, wview("w2a").bitcast(f16).rearrange(
                "(ko p m) -> p ko m", p=P, m=C))
            w2d = cp.tile([P, 4, C], f16)
            nc.sync.dma_start(w2d, wview("w2d").bitcast(f16).rearrange(
                "(ko p m) -> p ko m", p=P, m=C))
            g1t = cp.tile([P, 4], f32)
            nc.sync.dma_start(g1t, wview("g1t").rearrange("(p a) -> p a", p=P))
            b1t = cp.tile([P, 4], f32)
            nc.sync.dma_start(b1t, wview("b1t").rearrange("(p a) -> p a", p=P))
            g2t = cp.tile([P, 3], f32)
            nc.sync.dma_start(g2t, wview("g2t").rearrange("(p a) -> p a", p=P))
            b2t = cp.tile([P, 3], f32)
            nc.sync.dma_start(b2t, wview("b2t").rearrange("(p a) -> p a", p=P))
            sel1 = cp.tile([P, 4, 4], f32)
            nc.sync.dma_start(sel1, wview("sel1").rearrange(
                "(p a b) -> p a b", p=P, a=4))
            sel1t = cp.tile([4, 4, P], f32)
            nc.sync.dma_start(sel1t, wview("sel1t").rearrange(
                "(p a b) -> p a b", p=4, a=4))
            sel2 = cp.tile([P, 3, 4], f32)
            nc.sync.dma_start(sel2, wview("sel2").rearrange(
                "(p a b) -> p a b", p=P, a=3))
            sel2t = cp.tile([4, 3, P], f32)
            nc.sync.dma_start(sel2t, wview("sel2t").rearrange(
                "(p a b) -> p a b", p=4, a=3))
            epst = cp.tile([4, 1], f32)
            nc.vector.memset(epst, EPS)
            zt = cp.tile([P, 1], f32)
            nc.vector.memset(zt, 0.0)

            def knn_stage(nkeys, lq_t, rk_t):
                """Exact fp32 kNN scores + top-4.  Returns wl4 [P, 256] i16."""
                nch = nkeys // 512
                idx8 = sp.tile([P, 8, NT], dt.uint16, tag="idx8")
                for t in range(NT):
                    ndt = ta.tile([P, nkeys], f32, tag="ta")
                    for ch in range(nch):
                        ps = pnd.tile([P, 512], f32, tag="pnd")
                        nc.tensor.matmul(ps, lq_t[:, t * P:(t + 1) * P],
                                         rk_t[:, ch * 512:(ch + 1) * 512],
                                         start=True, stop=True)
                        nc.scalar.copy(ndt[:, ch * 512:(ch + 1) * 512], ps)
                    mx8 = sp.tile([P, 8], f32, tag="mx8")
                    nc.vector.max(out=mx8, in_=ndt)
                    nc.vector.max_index(out=idx8[:, :, t], in_max=mx8,
                                        in_values=ndt)
                idx4 = sp.tile([P, 4, NT], i16, tag="idx4")
                nc.vector.tensor_copy(idx4, idx8[:, 0:4, :].bitcast(i16))
                # wrapped gather list for ap_gather (i = j*1024 + q)
                wl4 = sp.tile([P, 4, 8, 8], i16, tag="wl4")  # [p, j, t, a]
                for a in range(8):
                    nc.sync.dma_start(
                        wl4[0:16, :, :, a], idx4[16 * a:16 * (a + 1)])
                wl4f = wl4.rearrange("p j t a -> p (j t a)")
                for g in range(1, 8):
                    nc.sync.dma_start(wl4f[16 * g:16 * (g + 1), :], wl4f[0:16, :])
                return wl4f

            def gn_prelu(n_c, maxed, sy, ssq, sel, selt, gt, bt, n_grp, out_t):
                """GroupNorm from raw per-partition sums + Prelu on maxed."""
                st2 = sp.tile([P, n_c, 2], f32, tag="st2")
                nc.vector.tensor_copy(st2[:, :, 0], sy)
                nc.vector.tensor_copy(st2[:, :, 1], ssq)
                psg = pst.tile([4, 2], f32, tag="psg")
                for c in range(n_c):
                    nc.tensor.matmul(psg, sel[:, c, :], st2[:, c, :],
                                     start=(c == 0), stop=(c == n_c - 1))
                gv = sp.tile([4, 2], f32, tag="gv")
                nc.scalar.mul(gv, psg, 1.0 / n_grp)
                msq = sp.tile([4, 1], f32, tag="msq")
                nc.vector.tensor_mul(msq, gv[:, 0:1], gv[:, 0:1])
                varg = sp.tile([4, 1], f32, tag="varg")
                nc.vector.tensor_sub(varg, gv[:, 1:2], msq)
                sd = sp.tile([4, 1], f32, tag="sd")
                nc.scalar.activation(sd, varg, AF.Sqrt, bias=epst[:], scale=1.0)
                mbv = sp.tile([4, 2], f32, tag="mbv")
                nc.vector.reciprocal(mbv[:, 1:2], sd)
                nc.vector.tensor_copy(mbv[:, 0:1], gv[:, 0:1])
                mv = sp.tile([P, n_c, 2], f32, tag="mv")
                for c in range(n_c):
                    psb = pst.tile([P, 2], f32, tag="psb")
                    nc.tensor.matmul(psb, selt[:, c, :], mbv, start=True, stop=True)
                    nc.scalar.copy(mv[:, c, :], psb)
                sv = sp.tile([P, n_c], f32, tag="sv")
                bv = sp.tile([P, n_c], f32, tag="bv")
                tmp = sp.tile([P, n_c], f32, tag="gtmp")
                nc.vector.tensor_mul(sv, gt, mv[:, :, 1])
                nc.vector.tensor_mul(tmp, mv[:, :, 0], sv)
                nc.vector.tensor_sub(bv, bt, tmp)
                for c in range(n_c):
                    nc.scalar.activation(
                        out_t[:, c, :], maxed[:, c, :], AF.Prelu,
                        bias=bv[:, c:c + 1], scale=sv[:, c:c + 1], alpha=ALPHA)

            def conv_plane(w, src, n_ko, m, out_c):
                """out_c[P, n] f32 <- sum_ko w[:, ko, m*P:(m+1)*P].T @ src[:, ko, :]"""
                n = src.shape[2]
                for ch in range(n // 512):
                    ps = pcv.tile([P, 512], f32, tag="pcv")
                    for ko in range(n_ko):
                        nc.tensor.matmul(ps, w[:, ko, m * P:(m + 1) * P],
                                         src[:, ko, ch * 512:(ch + 1) * 512],
                                         start=(ko == 0), stop=(ko == n_ko - 1))
                    nc.scalar.copy(out_c[:, ch * 512:(ch + 1) * 512], ps)

            def block(n_c, n_ko, wa, wd, src_u, src_v, wl4, nelems, sy, ssq, maxed):
                """Per-plane: conv U, gather, +V, stats, maxj.  V computed first."""
                vt = op.tile([P, n_c, GD], f16, tag="v")
                for m in range(n_c):
                    for ch in range(GD // 512):
                        ps = pcv.tile([P, 512], f32, tag="pcv")
                        for ko in range(n_ko):
                            nc.tensor.matmul(ps, wd[:, ko, m * P:(m + 1) * P],
                                             src_v[:, ko, ch * 512:(ch + 1) * 512],
                                             start=(ko == 0), stop=(ko == n_ko - 1))
                        nc.scalar.copy(vt[:, m, ch * 512:(ch + 1) * 512], ps)
                for c in range(n_c):
                    uc = ta.tile([P, nelems], f32, tag="ta")
                    conv_plane(wa, src_u, n_ko, c, uc)
                    ugc = tb.tile([P, 4 * GD], f32, tag="tb")
                    nc.gpsimd.ap_gather(
                        out_ap=ugc[:], in_ap=uc[:], idxs_ap=wl4,
                        channels=P, num_elems=nelems, d=1, num_idxs=4 * GD)
                    # y = ug + v (j-major), with sum accumulation
                    yc = sp.tile([P, 4, GD], f16, tag="yc")
                    nc.vector.scalar_tensor_tensor(
                        out=yc, in0=ugc.rearrange("p (j q) -> p j q", j=4),
                        scalar=0.0, in1=vt[:, c:c + 1, :].to_broadcast([P, 4, GD]),
                        op0=ALU.add, op1=ALU.add, accum_out=sy[:, c:c + 1])
                    # sum of squares via in-place ACT square
                    nc.scalar.activation(yc, yc, AF.Square, bias=zt[:], scale=1.0,
                                         accum_out=ssq[:, c:c + 1])
                    # max over j on ungathered-plus-v: max_j(ug) + v
                    ugr = ugc.rearrange("p (j q) -> p j q", j=4)
                    m0 = sp.tile([P, GD], f16, tag="m0")
                    m1 = sp.tile([P, GD], f16, tag="m1")
                    nc.vector.tensor_max(m0, ugr[:, 0, :], ugr[:, 1, :])
                    nc.vector.tensor_max(m1, ugr[:, 2, :], ugr[:, 3, :])
                    nc.vector.tensor_max(m0, m0, m1)
                    nc.vector.tensor_add(maxed[:, c, :], m0, vt[:, c, :])
                return vt

            for s in range(BC):
                # ---- per-sample loads ----
                lqo, _ = _SEC["lq"]
                lqt = op.tile([4, GD], f32, tag="lqt")
                nc.sync.dma_start(lqt, blob_d[0, lqo + s * 4 * GD:
                                              lqo + (s + 1) * 4 * GD]
                                  .rearrange("(r g) -> r g", r=4))
                r1o, _ = _SEC["rk1"]
                rk1t = op.tile([4, GS], f32, tag="rk1t")
                nc.sync.dma_start(rk1t, blob_d[0, r1o + s * 4 * GS:
                                               r1o + (s + 1) * 4 * GS]
                                  .rearrange("(r g) -> r g", r=4))
                r2o, _ = _SEC["rk2"]
                rk2t = op.tile([4, GD], f32, tag="rk2t")
                nc.sync.dma_start(rk2t, blob_d[0, r2o + s * 4 * GD:
                                               r2o + (s + 1) * 4 * GD]
                                  .rearrange("(r g) -> r g", r=4))
                fso, _ = _SEC["fs"]
                fs8 = tb.tile([P, 3, GS], i8, tag="tb")
                nc.sync.dma_start(
                    fs8, blob_d[0, fso + s * C * GS // 4:
                                fso + (s + 1) * C * GS // 4]
                    .bitcast(i8).rearrange("(ko p g) -> p ko g", p=P, g=GS))
                ssco, _ = _SEC["fssc"]
                fssc = sp.tile([P, 3], f32, tag="fssc")
                nc.sync.dma_start(fssc, blob_d[0, ssco + s * C:ssco + (s + 1) * C]
                                  .rearrange("(ko p) -> p ko", p=P))
                fs = bp.tile([P, 3, GS], f16, tag="fs_h")
                nc.vector.tensor_copy(fs, fs8)
                nc.vector.tensor_tensor(
                    out=fs, in0=fs, in1=fssc[:, :, None].to_broadcast([P, 3, GS]),
                    op=ALU.mult)
                fqo, _ = _SEC["fq"]
                fq8 = sp.tile([P, 3, GD], i8, tag="fq8")
                nc.sync.dma_start(
                    fq8, blob_d[0, fqo + s * C * GD // 4:
                                fqo + (s + 1) * C * GD // 4]
                    .bitcast(i8).rearrange("(ko p g) -> p ko g", p=P, g=GD))
                qsco, _ = _SEC["fqsc"]
                fqsc = sp.tile([P, 3], f32, tag="fqsc")
                nc.sync.dma_start(fqsc, blob_d[0, qsco + s * C:qsco + (s + 1) * C]
                                  .rearrange("(ko p) -> p ko", p=P))
                fq = op.tile([P, 3, GD], f16, tag="fq")
                nc.vector.tensor_copy(fq, fq8)
                nc.vector.tensor_tensor(
                    out=fq, in0=fq, in1=fqsc[:, :, None].to_broadcast([P, 3, GD]),
                    op=ALU.mult)

                # ---- kNN stage 1 & 2 ----
                wl4_1 = knn_stage(GS, lqt, rk1t)
                wl4_2 = knn_stage(GD, lqt, rk2t)

                # ---- block 1 ----
                sy1 = op.tile([P, 4], f32, tag="sy1")
                ssq1 = op.tile([P, 4], f32, tag="ssq1")
                maxed1 = op.tile([P, 4, GD], f16, tag="maxed")
                block(4, 3, w1a, w1d, fs, fq, wl4_1, GS, sy1, ssq1, maxed1)
                h = op.tile([P, 4, GD], f16, tag="fs_h")
                gn_prelu(4, maxed1, sy1, ssq1, sel1, sel1t, g1t, b1t,
                         P * 4 * GD, h)

                # ---- block 2 ----
                sy2 = op.tile([P, 3], f32, tag="sy2")
                ssq2 = op.tile([P, 3], f32, tag="ssq2")
                maxed2 = op.tile([P, 3, GD], f16, tag="maxed")
                block(3, 4, w2a, w2d, h, h, wl4_2, GD, sy2, ssq2, maxed2)
                outp = op.tile([P, 3, GD], f16, tag="outp")
                gn_prelu(3, maxed2, sy2, ssq2, sel2, sel2t, g2t, b2t,
                         96 * 4 * GD, outp)
                nc.sync.dma_start(
                    out_d[0, s * C * GD // 2:(s + 1) * C * GD // 2]
                    .bitcast(f16).rearrange("(c p g) -> p c g", p=P, g=GD),
                    outp)

    nc.compile()
    return nc


# ---------------- host runner ----------------
_STATE = None


class _State:
    pass


def _get_state():
    global _STATE
    if _STATE is not None:
        return _STATE
    import jax
    from jax.sharding import Mesh, PartitionSpec, NamedSharding
    from jax.experimental.shard_map import shard_map

    st = _State()
    st.nc = _build()
    nc = st.nc
    bass2jax.install_neuronx_cc_hook()
    partition_name = (nc.partition_id_tensor.name
                      if nc.partition_id_tensor else None)
    in_names = ["blob", "wblob", "out"]
    if partition_name:
        in_names.append(partition_name)
    out_avals = (jax.core.ShapedArray((1, OW), np.float32),)

    def _body(blob, wblob, outbuf):
        # `outbuf` is a dead parameter: the neuronx_cc_hook renames the BIR
        # "out" tensor to output0 (bound to the custom-call RESULT buffer),
        # so this operand's content is never read.  The kernel writes every
        # element of the result, so no pre-zeroing is needed either.
        operands = [blob, wblob, outbuf]
        if partition_name:
            operands.append(bass2jax.partition_id_tensor())
        return bass2jax._bass_exec_p.bind(
            *operands, out_avals=out_avals, in_names=tuple(in_names),
            out_names=("out",), lowering_input_output_aliases=(),
            sim_require_finite=True, sim_require_nnan=True, nc=nc)[0]

    devices = jax.devices()[:NCORE]
    mesh = Mesh(np.asarray(devices), ("core",))
    st.sharding = NamedSharding(mesh, PartitionSpec("core"))
    st.jitted = jax.jit(
        shard_map(_body, mesh=mesh,
                  in_specs=(PartitionSpec("core"),) * 3,
                  out_specs=PartitionSpec("core"), check_rep=False))
    st.dead_out = jax.device_put(np.zeros((NCORE, OW), np.float32),
                                 st.sharding)
    st.blob = np.zeros((NCORE, NW), np.float32)
    st.wblob = np.zeros((NCORE, WW), np.float32)
    st.wkey = None       # byte snapshot of (W1, W2, g1, b1, g2, b2)
    st.wdev = None       # device-resident weight blob
    # constant sections: selector matrices
    sel1 = np.zeros((P, 4, 4), np.float32)
    for c in range(4):
        for p in range(P):
            sel1[p, c, (c * P + p) // 128] = 1.0
    sel2 = np.zeros((P, 3, 4), np.float32)
    for c in range(3):
        for p in range(P):
            sel2[p, c, (c * P + p) // 96] = 1.0
    _fill(st.wblob, _WSEC, "sel1", sel1)
    _fill(st.wblob, _WSEC, "sel1t", np.ascontiguousarray(sel1.transpose(2, 1, 0)))
    _fill(st.wblob, _WSEC, "sel2", sel2)
    _fill(st.wblob, _WSEC, "sel2t", np.ascontiguousarray(sel2.transpose(2, 1, 0)))
    _STATE = st
    return st


def _fill(blob, secs, name, arr):
    o, w = secs[name]
    blob[:, o:o + w] = arr.reshape(1, -1).astype(np.float32)


def _view(blob, secs, name, shape, dtype=np.float32):
    """Per-core view of a blob section, reshaped to (NCORE, *shape)."""
    o, w = secs[name]
    v = blob[:, o:o + w]
    if dtype != np.float32:
        v = v.view(dtype)[:, :int(np.prod(shape))]
    return v.reshape(NCORE, *shape)


def _weights_changed(st, parts):
    key = b"".join(np.ascontiguousarray(p).tobytes() for p in parts)
    if st.wkey is not None and st.wkey == key:
        return False
    st.wkey = key
    return True


_POOL = None
_TMP = {}


def _tmp(key, shape):
    t = _TMP.get(key)
    if t is None:
        t = np.empty(shape, np.float32)
        _TMP[key] = t
    return t


def _quant_i8(x, q_out, sc_out):
    """Per-(core, sample, channel) symmetric int8 quantization, threaded
    over cores.

    sc_out layout per sample is (ko p): channel c = ko*128 + p maps to
    element ko*128 + p, matching the device's [p, ko] scale tile load.
    """
    global _POOL
    if _POOL is None:
        from concurrent.futures import ThreadPoolExecutor
        _POOL = ThreadPoolExecutor(NCORE)

    def one(c):
        xc = x[c]                                      # [BC, C, G]
        t = _tmp((xc.shape, c), xc.shape)
        amax = np.abs(xc, out=t).max(axis=-1)          # [BC, C]
        rsc = 127.0 / np.maximum(amax, 1e-30)
        np.multiply(xc, rsc[..., None], out=t)
        np.rint(t, out=t)
        np.copyto(q_out[c], t, casting="unsafe")
        sc_out[c] = (1.0 / rsc).reshape(sc_out.shape[1:])

    list(_POOL.map(one, range(NCORE)))


def kernel(**inputs):
    import jax
    st = _get_state()
    blob = st.blob

    inputs = {k: np.asarray(v) for k, v in inputs.items()}
    f = np.ascontiguousarray(inputs["f"], dtype=np.float32)
    f_q = np.ascontiguousarray(inputs["f_q"], dtype=np.float32)
    coor = np.ascontiguousarray(inputs["coor"], dtype=np.float32)
    coor_q = np.ascontiguousarray(inputs["coor_q"], dtype=np.float32)

    # big int8 payloads with per-(sample, channel) scales
    _quant_i8(f.reshape(NCORE, BC, C, GS),
              _view(blob, _SEC, "fs", (BC, C, GS), np.int8),
              _view(blob, _SEC, "fssc", (BC, C)))
    _quant_i8(f_q.reshape(NCORE, BC, C, GD),
              _view(blob, _SEC, "fq", (BC, C, GD), np.int8),
              _view(blob, _SEC, "fqsc", (BC, C)))

    # kNN rows (exact f32)
    k2s = (coor * coor).sum(axis=1)        # [16, GS] fp32, same order as ref
    k2q = (coor_q * coor_q).sum(axis=1)    # [16, GD]
    lq = _view(blob, _SEC, "lq", (BC, 4, GD))
    lq[:, :, 0:3, :] = coor_q.reshape(NCORE, BC, 3, GD)
    lq[:, :, 3, :] = 1.0
    rk1 = _view(blob, _SEC, "rk1", (BC, 4, GS))
    rk1[:, :, 0:3, :] = 2.0 * coor.reshape(NCORE, BC, 3, GS)
    rk1[:, :, 3, :] = -k2s.reshape(NCORE, BC, GS)
    rk2 = _view(blob, _SEC, "rk2", (BC, 4, GD))
    rk2[:, :, 0:3, :] = 2.0 * coor_q.reshape(NCORE, BC, 3, GD)
    rk2[:, :, 3, :] = -k2q.reshape(NCORE, BC, GD)

    dev = jax.device_put(blob, st.sharding)   # async; overlaps weight check

    # weights (device-resident unless changed)
    W1 = inputs["W1"].astype(np.float32, copy=False)
    W2 = inputs["W2"].astype(np.float32, copy=False)
    g1 = inputs["g1"].astype(np.float32, copy=False)
    b1 = inputs["b1"].astype(np.float32, copy=False)
    g2 = inputs["g2"].astype(np.float32, copy=False)
    b2 = inputs["b2"].astype(np.float32, copy=False)
    if st.wdev is None or _weights_changed(st, (W1, W2, g1, b1, g2, b2)):
        W1a, W1b = W1[:, :C], W1[:, C:]
        W2a, W2b = W2[:, :512], W2[:, 512:]
        wb = st.wblob
        np.copyto(_view(wb, _WSEC, "w1a", (C, 512), np.float16), W1a.T[None])
        np.copyto(_view(wb, _WSEC, "w1d", (C, 512), np.float16),
                  (W1b - W1a).T[None])
        np.copyto(_view(wb, _WSEC, "w2a", (512, C), np.float16), W2a.T[None])
        np.copyto(_view(wb, _WSEC, "w2d", (512, C), np.float16),
                  (W2b - W2a).T[None])
        _fill(wb, _WSEC, "g1t", np.ascontiguousarray(g1.reshape(4, P).T))
        _fill(wb, _WSEC, "b1t", np.ascontiguousarray(b1.reshape(4, P).T))
        _fill(wb, _WSEC, "g2t", np.ascontiguousarray(g2.reshape(3, P).T))
        _fill(wb, _WSEC, "b2t", np.ascontiguousarray(b2.reshape(3, P).T))
        st.wdev = jax.device_put(wb, st.sharding)

    out = st.jitted(dev, st.wdev, st.dead_out)
    res = np.asarray(out)                     # [8, OW] f32 container
    o16 = res.reshape(NCORE, -1).view(np.float16)[:, :BC * C * GD]
    return o16.reshape(B, C, GD).astype(np.float32)


# revision 4
# speedup vs baseline: 1.0425x; 1.0425x over previous
"""DGCNN_Propagation Trainium2 Bass kernel, v3 (int8 transport).

Data-parallel over batch: 16 samples -> 8 NeuronCores, 2 samples/core.

Key design points (driven by axon-tunnel profiling):
  - ONE packed f32 dynamic input blob per core + ONE f32 weight blob
    (device-resident across calls, revalidated by byte-compare) + ONE f32
    output blob: each extra jax array costs ~75ms of RPC latency per call,
    and non-f32 IO dtypes add fixed per-exec penalties (int16 outputs:
    ~130ms each!).  All f16/fp8 payloads are bitcast views inside f32 blobs.
  - f / f_q ship as per-(sample,channel) scaled int8 and are dequantized
    to f16 on device before the matmuls (weights stay f16).  ~0.9% RMS
    quantization noise vs fp8's 3.6%, at the same 1 byte/value.
  - kNN scores computed EXACTLY in fp32 on the TensorE (4-row matmul:
    2q.k - |k|^2; per-query constant q^2 dropped as it can't change
    ranking).  No coarse/refine split, no dma_gather, no kr tables.
  - Conv folding: W @ [gather(f)-xq; xq] == gather(Wa @ f) + (Wb-Wa) @ xq,
    so matmuls run on ungathered data and the gather (gpsimd ap_gather)
    runs per conv-output channel plane.
  - GroupNorm via per-partition accumulators + tiny selector matmuls;
    max-over-k pulled before the monotone affine; affine+LeakyReLU fused
    into one ACT Prelu op.  Activations f16.
"""

import numpy as np

import concourse.bass as bass
import concourse.bacc as bacc
import concourse.mybir as mybir
from concourse import bass2jax
from concourse.tile import TileContext

dt = mybir.dt
AF = mybir.ActivationFunctionType
ALU = mybir.AluOpType

P = 128
B, C, GS, GD, K = 16, 384, 4096, 1024, 4
BC = 2              # samples per core
NCORE = 8
NT = GD // P        # 8 query tiles
EPS = 1e-5
ALPHA = 0.2

f16 = dt.float16
f32 = dt.float32
i8 = dt.int8
i16 = dt.int16

# ---------------- blob layouts (f32 words, per core) ----------------


def _mk_layout(specs):
    off, sec = 0, {}
    for name, words in specs:
        sec[name] = (off, words)
        off += (words + 127) & ~127
    return sec, (off + 511) & ~511


_SEC, NW = _mk_layout([
    ("fs", BC * C * GS // 4),     # i8 [BC, 384, 4096], per-channel scaled
    ("fq", BC * C * GD // 4),     # i8 [BC, 384, 1024], per-channel scaled
    ("fssc", BC * C),             # f32 [BC, 3, 128] dequant scales (ko p)
    ("fqsc", BC * C),             # f32 [BC, 3, 128]
    ("lq", BC * 4 * GD),          # f32 [BC, 4, 1024]  rows: q0,q1,q2,1
    ("rk1", BC * 4 * GS),         # f32 [BC, 4, 4096]  rows: 2k0,2k1,2k2,-k2
    ("rk2", BC * 4 * GD),         # f32 [BC, 4, 1024]
])

_WSEC, WW = _mk_layout([
    ("w1a", C * 512 // 2),        # f16 [384, 512]
    ("w1d", C * 512 // 2),
    ("w2a", 512 * C // 2),        # f16 [512, 384]
    ("w2d", 512 * C // 2),
    ("g1t", P * 4),               # f32 [128, 4]
    ("b1t", P * 4),
    ("g2t", P * 3),
    ("b2t", P * 3),
    ("sel1", P * 4 * 4),
    ("sel1t", 4 * 4 * P),
    ("sel2", P * 3 * 4),
    ("sel2t", 4 * 3 * P),
])

OW = BC * C * GD // 2             # output blob words per core (f16 payload)


def _build():
    nc = bacc.Bacc("TRN2", target_bir_lowering=False, debug=False,
                   num_devices=NCORE)

    blob_d = nc.dram_tensor("blob", [1, NW], f32, kind="ExternalInput")
    wblob_d = nc.dram_tensor("wblob", [1, WW], f32, kind="ExternalInput")
    out_d = nc.dram_tensor("out", [1, OW], f32, kind="ExternalOutput")

    def bview(name):
        o, w = _SEC[name]
        return blob_d[0, o:o + w]

    def wview(name):
        o, w = _WSEC[name]
        return wblob_d[0, o:o + w]

    with TileContext(nc) as tc:
        with (
            tc.tile_pool(name="const", bufs=1) as cp,
            tc.tile_pool(name="big", bufs=1) as bp,
            tc.tile_pool(name="one", bufs=1) as op,
            tc.tile_pool(name="ta", bufs=2) as ta,    # ndt / u1c / u2c
            tc.tile_pool(name="tb", bufs=2) as tb,    # ug1c / ug2c / fp8 staging
            tc.tile_pool(name="sm", bufs=2) as sp,
            tc.tile_pool(name="pnd", bufs=2, space="PSUM") as pnd,
            tc.tile_pool(name="pcv", bufs=2, space="PSUM") as pcv,
            tc.tile_pool(name="pst", bufs=2, space="PSUM") as pst,
        ):
            # ---- constants (shared by both samples) ----
            w1a = cp.tile([P, 3, 512], f16)
            nc.sync.dma_start(w1a, wview("w1a").bitcast(f16).rearrange(
                "(ko p m) -> p ko m", p=P, m=512))
            w1d = cp.tile([P, 3, 512], f16)
            nc.sync.dma_start(w1d, wview("w1d").bitcast(f16).rearrange(
                "(ko p m) -> p ko m", p=P, m=512))
            w2a = cp.tile([P, 4, C], f16)
            nc.sync.dma_start(w2a, wview("w2a").bitcast(f16).rearrange(
                "(ko p m) -> p ko m", p=P, m=C))
            w2d = cp.tile([P, 4, C], f16)
            nc.sync.dma_start(w2d, wview("w2d").bitcast(f16).rearrange(
                "(ko p m) -> p ko m", p=P, m=C))
            g1t = cp.tile([P, 4], f32)
            nc.sync.dma_start(g1t, wview("g1t").rearrange("(p a) -> p a", p=P))
            b1t = cp.tile([P, 4], f32)
            nc.sync.dma_start(b1t, wview("b1t").rearrange("(p a) -> p a", p=P))
            g2t = cp.tile([P, 3], f32)
            nc.sync.dma_start(g2t, wview("g2t").rearrange("(p a) -> p a", p=P))
            b2t = cp.tile([P, 3], f32)
            nc.sync.dma_start(b2t, wview("b2t").rearrange("(p a) -> p a", p=P))
            sel1 = cp.tile([P, 4, 4], f32)
            nc.sync.dma_start(sel1, wview("sel1").rearrange(
                "(p a b) -> p a b", p=P, a=4))
            sel1t = cp.tile([4, 4, P], f32)
            nc.sync.dma_start(sel1t, wview("sel1t").rearrange(
                "(p a b) -> p a b", p=4, a=4))
            sel2 = cp.tile([P, 3, 4], f32)
            nc.sync.dma_start(sel2, wview("sel2").rearrange(
                "(p a b) -> p a b", p=P, a=3))
            sel2t = cp.tile([4, 3, P], f32)
            nc.sync.dma_start(sel2t, wview("sel2t").rearrange(
                "(p a b) -> p a b", p=4, a=3))
            epst = cp.tile([4, 1], f32)
            nc.vector.memset(epst, EPS)
            zt = cp.tile([P, 1], f32)
            nc.vector.memset(zt, 0.0)

            def knn_stage(nkeys, lq_t, rk_t):
                """Exact fp32 kNN scores + top-4.  Returns wl4 [P, 256] i16."""
                nch = nkeys // 512
                idx8 = sp.tile([P, 8, NT], dt.uint16, tag="idx8")
                for t in range(NT):
                    ndt = ta.tile([P, nkeys], f32, tag="ta")
                    for ch in range(nch):
                        ps = pnd.tile([P, 512], f32, tag="pnd")
                        nc.tensor.matmul(ps, lq_t[:, t * P:(t + 1) * P],
                                         rk_t[:, ch * 512:(ch + 1) * 512],
                                         start=True, stop=True)
                        nc.scalar.copy(ndt[:, ch * 512:(ch + 1) * 512], ps)
                    mx8 = sp.tile([P, 8], f32, tag="mx8")
                    nc.vector.max(out=mx8, in_=ndt)
                    nc.vector.max_index(out=idx8[:, :, t], in_max=mx8,
                                        in_values=ndt)
                idx4 = sp.tile([P, 4, NT], i16, tag="idx4")
                nc.vector.tensor_copy(idx4, idx8[:, 0:4, :].bitcast(i16))
                # wrapped gather list for ap_gather (i = j*1024 + q)
                wl4 = sp.tile([P, 4, 8, 8], i16, tag="wl4")  # [p, j, t, a]
                for a in range(8):
                    nc.sync.dma_start(
                        wl4[0:16, :, :, a], idx4[16 * a:16 * (a + 1)])
                wl4f = wl4.rearrange("p j t a -> p (j t a)")
                for g in range(1, 8):
                    nc.sync.dma_start(wl4f[16 * g:16 * (g + 1), :], wl4f[0:16, :])
                return wl4f

            def gn_prelu(n_c, maxed, sy, ssq, sel, selt, gt, bt, n_grp, out_t):
                """GroupNorm from raw per-partition sums + Prelu on maxed."""
                st2 = sp.tile([P, n_c, 2], f32, tag="st2")
                nc.vector.tensor_copy(st2[:, :, 0], sy)
                nc.vector.tensor_copy(st2[:, :, 1], ssq)
                psg = pst.tile([4, 2], f32, tag="psg")
                for c in range(n_c):
                    nc.tensor.matmul(psg, sel[:, c, :], st2[:, c, :],
                                     start=(c == 0), stop=(c == n_c - 1))
                gv = sp.tile([4, 2], f32, tag="gv")
                nc.scalar.mul(gv, psg, 1.0 / n_grp)
                msq = sp.tile([4, 1], f32, tag="msq")
                nc.vector.tensor_mul(msq, gv[:, 0:1], gv[:, 0:1])
                varg = sp.tile([4, 1], f32, tag="varg")
                nc.vector.tensor_sub(varg, gv[:, 1:2], msq)
                sd = sp.tile([4, 1], f32, tag="sd")
                nc.scalar.activation(sd, varg, AF.Sqrt, bias=epst[:], scale=1.0)
                mbv = sp.tile([4, 2], f32, tag="mbv")
                nc.vector.reciprocal(mbv[:, 1:2], sd)
                nc.vector.tensor_copy(mbv[:, 0:1], gv[:, 0:1])
                mv = sp.tile([P, n_c, 2], f32, tag="mv")
                for c in range(n_c):
                    psb = pst.tile([P, 2], f32, tag="psb")
                    nc.tensor.matmul(psb, selt[:, c, :], mbv, start=True, stop=True)
                    nc.scalar.copy(mv[:, c, :], psb)
                sv = sp.tile([P, n_c], f32, tag="sv")
                bv = sp.tile([P, n_c], f32, tag="bv")
                tmp = sp.tile([P, n_c], f32, tag="gtmp")
                nc.vector.tensor_mul(sv, gt, mv[:, :, 1])
                nc.vector.tensor_mul(tmp, mv[:, :, 0], sv)
                nc.vector.tensor_sub(bv, bt, tmp)
                for c in range(n_c):
                    nc.scalar.activation(
                        out_t[:, c, :], maxed[:, c, :], AF.Prelu,
                        bias=bv[:, c:c + 1], scale=sv[:, c:c + 1], alpha=ALPHA)

            def conv_plane(w, src, n_ko, m, out_c):
                """out_c[P, n] f32 <- sum_ko w[:, ko, m*P:(m+1)*P].T @ src[:, ko, :]"""
                n = src.shape[2]
                for ch in range(n // 512):
                    ps = pcv.tile([P, 512], f32, tag="pcv")
                    for ko in range(n_ko):
                        nc.tensor.matmul(ps, w[:, ko, m * P:(m + 1) * P],
                                         src[:, ko, ch * 512:(ch + 1) * 512],
                                         start=(ko == 0), stop=(ko == n_ko - 1))
                    nc.scalar.copy(out_c[:, ch * 512:(ch + 1) * 512], ps)

            def block(n_c, n_ko, wa, wd, src_u, src_v, wl4, nelems, sy, ssq, maxed):
                """Per-plane: conv U, gather, +V, stats, maxj.  V computed first."""
                vt = op.tile([P, n_c, GD], f16, tag="v")
                for m in range(n_c):
                    for ch in range(GD // 512):
                        ps = pcv.tile([P, 512], f32, tag="pcv")
                        for ko in range(n_ko):
                            nc.tensor.matmul(ps, wd[:, ko, m * P:(m + 1) * P],
                                             src_v[:, ko, ch * 512:(ch + 1) * 512],
                                             start=(ko == 0), stop=(ko == n_ko - 1))
                        nc.scalar.copy(vt[:, m, ch * 512:(ch + 1) * 512], ps)
                for c in range(n_c):
                    uc = ta.tile([P, nelems], f32, tag="ta")
                    conv_plane(wa, src_u, n_ko, c, uc)
                    ugc = tb.tile([P, 4 * GD], f32, tag="tb")
                    nc.gpsimd.ap_gather(
                        out_ap=ugc[:], in_ap=uc[:], idxs_ap=wl4,
                        channels=P, num_elems=nelems, d=1, num_idxs=4 * GD)
                    # y = ug + v (j-major), with sum accumulation
                    yc = sp.tile([P, 4, GD], f16, tag="yc")
                    nc.vector.scalar_tensor_tensor(
                        out=yc, in0=ugc.rearrange("p (j q) -> p j q", j=4),
                        scalar=0.0, in1=vt[:, c:c + 1, :].to_broadcast([P, 4, GD]),
                        op0=ALU.add, op1=ALU.add, accum_out=sy[:, c:c + 1])
                    # sum of squares via in-place ACT square
                    nc.scalar.activation(yc, yc, AF.Square, bias=zt[:], scale=1.0,
                                         accum_out=ssq[:, c:c + 1])
                    # max over j on ungathered-plus-v: max_j(ug) + v
                    ugr = ugc.rearrange("p (j q) -> p j q", j=4)
                    m0 = sp.tile([P, GD], f16, tag="m0")
                    m1 = sp.tile([P, GD], f16, tag="m1")
                    nc.vector.tensor_max(m0, ugr[:, 0, :], ugr[:, 1, :])
                    nc.vector.tensor_max(m1, ugr[:, 2, :], ugr[:, 3, :])
                    nc.vector.tensor_max(m0, m0, m1)
                    nc.vector.tensor_add(maxed[:, c, :], m0, vt[:, c, :])
                return vt

            for s in range(BC):
                # ---- per-sample loads ----
                lqo, _ = _SEC["lq"]
                lqt = op.tile([4, GD], f32, tag="lqt")
                nc.sync.dma_start(lqt, blob_d[0, lqo + s * 4 * GD:
                                              lqo + (s + 1) * 4 * GD]
                                  .rearrange("(r g) -> r g", r=4))
                r1o, _ = _SEC["rk1"]
                rk1t = op.tile([4, GS], f32, tag="rk1t")
                nc.sync.dma_start(rk1t, blob_d[0, r1o + s * 4 * GS:
                                               r1o + (s + 1) * 4 * GS]
                                  .rearrange("(r g) -> r g", r=4))
                r2o, _ = _SEC["rk2"]
                rk2t = op.tile([4, GD], f32, tag="rk2t")
                nc.sync.dma_start(rk2t, blob_d[0, r2o + s * 4 * GD:
                                               r2o + (s + 1) * 4 * GD]
                                  .rearrange("(r g) -> r g", r=4))
                fso, _ = _SEC["fs"]
                fs8 = tb.tile([P, 3, GS], i8, tag="tb")
                nc.sync.dma_start(
                    fs8, blob_d[0, fso + s * C * GS // 4:
                                fso + (s + 1) * C * GS // 4]
                    .bitcast(i8).rearrange("(ko p g) -> p ko g", p=P, g=GS))
                ssco, _ = _SEC["fssc"]
                fssc = sp.tile([P, 3], f32, tag="fssc")
                nc.sync.dma_start(fssc, blob_d[0, ssco + s * C:ssco + (s + 1) * C]
                                  .rearrange("(ko p) -> p ko", p=P))
                fs = bp.tile([P, 3, GS], f16, tag="fs_h")
                nc.vector.tensor_copy(fs, fs8)
                nc.vector.tensor_tensor(
                    out=fs, in0=fs, in1=fssc[:, :, None].to_broadcast([P, 3, GS]),
                    op=ALU.mult)
                fqo, _ = _SEC["fq"]
                fq8 = sp.tile([P, 3, GD], i8, tag="fq8")
                nc.sync.dma_start(
                    fq8, blob_d[0, fqo + s * C * GD // 4:
                                fqo + (s + 1) * C * GD // 4]
                    .bitcast(i8).rearrange("(ko p g) -> p ko g", p=P, g=GD))
                qsco, _ = _SEC["fqsc"]
                fqsc = sp.tile([P, 3], f32, tag="fqsc")
                nc.sync.dma_start(fqsc, blob_d[0, qsco + s * C:qsco + (s + 1) * C]
                                  .rearrange("(ko p) -> p ko", p=P))
                fq = op.tile([P, 3, GD], f16, tag="fq")
                nc.vector.tensor_copy(fq, fq8)
                nc.vector.tensor_tensor(
                    out=fq, in0=fq, in1=fqsc[:, :, None].to_broadcast([P, 3, GD]),
                    op=ALU.mult)

                # ---- kNN stage 1 & 2 ----
                wl4_1 = knn_stage(GS, lqt, rk1t)
                wl4_2 = knn_stage(GD, lqt, rk2t)

                # ---- block 1 ----
                sy1 = op.tile([P, 4], f32, tag="sy1")
                ssq1 = op.tile([P, 4], f32, tag="ssq1")
                maxed1 = op.tile([P, 4, GD], f16, tag="maxed")
                block(4, 3, w1a, w1d, fs, fq, wl4_1, GS, sy1, ssq1, maxed1)
                h = op.tile([P, 4, GD], f16, tag="fs_h")
                gn_prelu(4, maxed1, sy1, ssq1, sel1, sel1t, g1t, b1t,
                         P * 4 * GD, h)

                # ---- block 2 ----
                sy2 = op.tile([P, 3], f32, tag="sy2")
                ssq2 = op.tile([P, 3], f32, tag="ssq2")
                maxed2 = op.tile([P, 3, GD], f16, tag="maxed")
                block(3, 4, w2a, w2d, h, h, wl4_2, GD, sy2, ssq2, maxed2)
                outp = op.tile([P, 3, GD], f16, tag="outp")
                gn_prelu(3, maxed2, sy2, ssq2, sel2, sel2t, g2t, b2t,
                         96 * 4 * GD, outp)
                nc.sync.dma_start(
                    out_d[0, s * C * GD // 2:(s + 1) * C * GD // 2]
                    .bitcast(f16).rearrange("(c p g) -> p c g", p=P, g=GD),
                    outp)

    nc.compile()
    return nc


# ---------------- host runner ----------------
_STATE = None


class _State:
    pass


def _get_state():
    global _STATE
    if _STATE is not None:
        return _STATE
    import jax
    from jax.sharding import Mesh, PartitionSpec, NamedSharding
    from jax.experimental.shard_map import shard_map

    st = _State()
    st.nc = _build()
    nc = st.nc
    bass2jax.install_neuronx_cc_hook()
    partition_name = (nc.partition_id_tensor.name
                      if nc.partition_id_tensor else None)
    in_names = ["blob", "wblob", "out"]
    if partition_name:
        in_names.append(partition_name)
    out_avals = (jax.core.ShapedArray((1, OW), np.float32),)

    def _body(blob, wblob, outbuf):
        # `outbuf` is a dead parameter: the neuronx_cc_hook renames the BIR
        # "out" tensor to output0 (bound to the custom-call RESULT buffer),
        # so this operand's content is never read.  The kernel writes every
        # element of the result, so no pre-zeroing is needed either.
        operands = [blob, wblob, outbuf]
        if partition_name:
            operands.append(bass2jax.partition_id_tensor())
        return bass2jax._bass_exec_p.bind(
            *operands, out_avals=out_avals, in_names=tuple(in_names),
            out_names=("out",), lowering_input_output_aliases=(),
            sim_require_finite=True, sim_require_nnan=True, nc=nc)[0]

    devices = jax.devices()[:NCORE]
    mesh = Mesh(np.asarray(devices), ("core",))
    st.sharding = NamedSharding(mesh, PartitionSpec("core"))
    st.jitted = jax.jit(
        shard_map(_body, mesh=mesh,
                  in_specs=(PartitionSpec("core"),) * 3,
                  out_specs=PartitionSpec("core"), check_rep=False))
    st.dead_out = jax.device_put(np.zeros((NCORE, OW), np.float32),
                                 st.sharding)
    st.blob = np.zeros((NCORE, NW), np.float32)
    st.wblob = np.zeros((NCORE, WW), np.float32)
    st.wkey = None       # byte snapshot of (W1, W2, g1, b1, g2, b2)
    st.wdev = None       # device-resident weight blob
    # constant sections: selector matrices
    sel1 = np.zeros((P, 4, 4), np.float32)
    for c in range(4):
        for p in range(P):
            sel1[p, c, (c * P + p) // 128] = 1.0
    sel2 = np.zeros((P, 3, 4), np.float32)
    for c in range(3):
        for p in range(P):
            sel2[p, c, (c * P + p) // 96] = 1.0
    _fill(st.wblob, _WSEC, "sel1", sel1)
    _fill(st.wblob, _WSEC, "sel1t", np.ascontiguousarray(sel1.transpose(2, 1, 0)))
    _fill(st.wblob, _WSEC, "sel2", sel2)
    _fill(st.wblob, _WSEC, "sel2t", np.ascontiguousarray(sel2.transpose(2, 1, 0)))
    _STATE = st
    return st


def _fill(blob, secs, name, arr):
    o, w = secs[name]
    blob[:, o:o + w] = arr.reshape(1, -1).astype(np.float32)


def _view(blob, secs, name, shape, dtype=np.float32):
    """Per-core view of a blob section, reshaped to (NCORE, *shape)."""
    o, w = secs[name]
    v = blob[:, o:o + w]
    if dtype != np.float32:
        v = v.view(dtype)[:, :int(np.prod(shape))]
    return v.reshape(NCORE, *shape)


def _weights_changed(st, parts):
    key = b"".join(np.ascontiguousarray(p).tobytes() for p in parts)
    if st.wkey is not None and st.wkey == key:
        return False
    st.wkey = key
    return True


_POOL = None
_TMP = {}


def _tmp(key, shape):
    t = _TMP.get(key)
    if t is None:
        t = np.empty(shape, np.float32)
        _TMP[key] = t
    return t


def _quant_i8(x, q_out, sc_out):
    """Per-(core, sample, channel) symmetric int8 quantization, threaded
    over cores.

    sc_out layout per sample is (ko p): channel c = ko*128 + p maps to
    element ko*128 + p, matching the device's [p, ko] scale tile load.
    """
    global _POOL
    if _POOL is None:
        from concurrent.futures import ThreadPoolExecutor
        _POOL = ThreadPoolExecutor(NCORE)

    def one(c):
        xc = x[c]                                      # [BC, C, G]
        t = _tmp((xc.shape, c), xc.shape)
        amax = np.maximum(xc.max(axis=-1), -xc.min(axis=-1))   # [BC, C]
        rsc = 127.0 / np.maximum(amax, 1e-30)
        np.multiply(xc, rsc[..., None], out=t)
        np.rint(t, out=t)
        np.copyto(q_out[c], t, casting="unsafe")
        sc_out[c] = (1.0 / rsc).reshape(sc_out.shape[1:])

    list(_POOL.map(one, range(NCORE)))


def kernel(**inputs):
    import jax
    st = _get_state()
    blob = st.blob

    inputs = {k: np.asarray(v) for k, v in inputs.items()}
    f = np.ascontiguousarray(inputs["f"], dtype=np.float32)
    f_q = np.ascontiguousarray(inputs["f_q"], dtype=np.float32)
    coor = np.ascontiguousarray(inputs["coor"], dtype=np.float32)
    coor_q = np.ascontiguousarray(inputs["coor_q"], dtype=np.float32)

    # big int8 payloads with per-(sample, channel) scales
    _quant_i8(f.reshape(NCORE, BC, C, GS),
              _view(blob, _SEC, "fs", (BC, C, GS), np.int8),
              _view(blob, _SEC, "fssc", (BC, C)))
    _quant_i8(f_q.reshape(NCORE, BC, C, GD),
              _view(blob, _SEC, "fq", (BC, C, GD), np.int8),
              _view(blob, _SEC, "fqsc", (BC, C)))

    # kNN rows (exact f32)
    k2s = (coor * coor).sum(axis=1)        # [16, GS] fp32, same order as ref
    k2q = (coor_q * coor_q).sum(axis=1)    # [16, GD]
    lq = _view(blob, _SEC, "lq", (BC, 4, GD))
    lq[:, :, 0:3, :] = coor_q.reshape(NCORE, BC, 3, GD)
    lq[:, :, 3, :] = 1.0
    rk1 = _view(blob, _SEC, "rk1", (BC, 4, GS))
    rk1[:, :, 0:3, :] = 2.0 * coor.reshape(NCORE, BC, 3, GS)
    rk1[:, :, 3, :] = -k2s.reshape(NCORE, BC, GS)
    rk2 = _view(blob, _SEC, "rk2", (BC, 4, GD))
    rk2[:, :, 0:3, :] = 2.0 * coor_q.reshape(NCORE, BC, 3, GD)
    rk2[:, :, 3, :] = -k2q.reshape(NCORE, BC, GD)

    dev = jax.device_put(blob, st.sharding)   # async; overlaps weight check

    # weights (device-resident unless changed)
    W1 = inputs["W1"].astype(np.float32, copy=False)
    W2 = inputs["W2"].astype(np.float32, copy=False)
    g1 = inputs["g1"].astype(np.float32, copy=False)
    b1 = inputs["b1"].astype(np.float32, copy=False)
    g2 = inputs["g2"].astype(np.float32, copy=False)
    b2 = inputs["b2"].astype(np.float32, copy=False)
    if st.wdev is None or _weights_changed(st, (W1, W2, g1, b1, g2, b2)):
        W1a, W1b = W1[:, :C], W1[:, C:]
        W2a, W2b = W2[:, :512], W2[:, 512:]
        wb = st.wblob
        np.copyto(_view(wb, _WSEC, "w1a", (C, 512), np.float16), W1a.T[None])
        np.copyto(_view(wb, _WSEC, "w1d", (C, 512), np.float16),
                  (W1b - W1a).T[None])
        np.copyto(_view(wb, _WSEC, "w2a", (512, C), np.float16), W2a.T[None])
        np.copyto(_view(wb, _WSEC, "w2d", (512, C), np.float16),
                  (W2b - W2a).T[None])
        _fill(wb, _WSEC, "g1t", np.ascontiguousarray(g1.reshape(4, P).T))
        _fill(wb, _WSEC, "b1t", np.ascontiguousarray(b1.reshape(4, P).T))
        _fill(wb, _WSEC, "g2t", np.ascontiguousarray(g2.reshape(3, P).T))
        _fill(wb, _WSEC, "b2t", np.ascontiguousarray(b2.reshape(3, P).T))
        st.wdev = jax.device_put(wb, st.sharding)

    out = st.jitted(dev, st.wdev, st.dead_out)
    res = np.asarray(out)                     # [8, OW] f32 container
    o16 = res.reshape(NCORE, -1).view(np.float16)[:, :BC * C * GD]
    return o16.reshape(B, C, GD).astype(np.float32)


# revision 9
# speedup vs baseline: 1.1940x; 1.1453x over previous
"""DGCNN_Propagation Trainium2 Bass kernel, v3 (int8 transport).

Data-parallel over batch: 16 samples -> 8 NeuronCores, 2 samples/core.

Key design points (driven by axon-tunnel profiling):
  - ONE packed f32 dynamic input blob per core + ONE f32 weight blob
    (device-resident across calls, revalidated by byte-compare) + ONE f32
    output blob: each extra jax array costs ~75ms of RPC latency per call,
    and non-f32 IO dtypes add fixed per-exec penalties (int16 outputs:
    ~130ms each!).  All f16/fp8 payloads are bitcast views inside f32 blobs.
  - f / f_q ship as per-(sample,channel) scaled int8 and are dequantized
    to f16 on device before the matmuls (weights stay f16).  ~0.9% RMS
    quantization noise vs fp8's 3.6%, at the same 1 byte/value.
  - kNN scores computed EXACTLY in fp32 on the TensorE (4-row matmul:
    2q.k - |k|^2; per-query constant q^2 dropped as it can't change
    ranking).  No coarse/refine split, no dma_gather, no kr tables.
  - Conv folding: W @ [gather(f)-xq; xq] == gather(Wa @ f) + (Wb-Wa) @ xq,
    so matmuls run on ungathered data and the gather (gpsimd ap_gather)
    runs per conv-output channel plane.
  - GroupNorm via per-partition accumulators + tiny selector matmuls;
    max-over-k pulled before the monotone affine; affine+LeakyReLU fused
    into one ACT Prelu op.  Activations f16.
"""

import numpy as np

import concourse.bass as bass
import concourse.bacc as bacc
import concourse.mybir as mybir
from concourse import bass2jax
from concourse.tile import TileContext

dt = mybir.dt
AF = mybir.ActivationFunctionType
ALU = mybir.AluOpType

P = 128
B, C, GS, GD, K = 16, 384, 4096, 1024, 4
BC = 2              # samples per core
NCORE = 8
NT = GD // P        # 8 query tiles
EPS = 1e-5
ALPHA = 0.2

f16 = dt.float16
f32 = dt.float32
i8 = dt.int8
i16 = dt.int16

# ---------------- blob layouts (f32 words, per core) ----------------


def _mk_layout(specs):
    off, sec = 0, {}
    for name, words in specs:
        sec[name] = (off, words)
        off += (words + 127) & ~127
    return sec, (off + 511) & ~511


_SEC, NW = _mk_layout([
    ("fs", BC * C * GS // 4),     # i8 [BC, 384, 4096], per-channel scaled
    ("fq", BC * C * GD // 4),     # i8 [BC, 384, 1024], per-channel scaled
    ("fssc", BC * C),             # f32 [BC, 3, 128] dequant scales (ko p)
    ("fqsc", BC * C),             # f32 [BC, 3, 128]
    ("lq", BC * 4 * GD),          # f32 [BC, 4, 1024]  rows: q0,q1,q2,1
    ("rk1", BC * 4 * GS),         # f32 [BC, 4, 4096]  rows: 2k0,2k1,2k2,-k2
    ("rk2", BC * 4 * GD),         # f32 [BC, 4, 1024]
])

_WSEC, WW = _mk_layout([
    ("w1a", C * 512 // 2),        # f16 [384, 512]
    ("w1d", C * 512 // 2),
    ("w2a", 512 * C // 2),        # f16 [512, 384]
    ("w2d", 512 * C // 2),
    ("g1t", P * 4),               # f32 [128, 4]
    ("b1t", P * 4),
    ("g2t", P * 3),
    ("b2t", P * 3),
    ("sel1", P * 4 * 4),
    ("sel1t", 4 * 4 * P),
    ("sel2", P * 3 * 4),
    ("sel2t", 4 * 3 * P),
])

QW = P * 3 * GD // 4              # int8 output payload words per sample
SWRD = QW + 512                   # per-sample stride incl. f32 scales
OW = BC * SWRD                    # output blob words per core


def _build():
    nc = bacc.Bacc("TRN2", target_bir_lowering=False, debug=False,
                   num_devices=NCORE)

    blob_d = nc.dram_tensor("blob", [1, NW], f32, kind="ExternalInput")
    wblob_d = nc.dram_tensor("wblob", [1, WW], f32, kind="ExternalInput")
    out_d = nc.dram_tensor("out", [1, OW], f32, kind="ExternalOutput")

    def bview(name):
        o, w = _SEC[name]
        return blob_d[0, o:o + w]

    def wview(name):
        o, w = _WSEC[name]
        return wblob_d[0, o:o + w]

    with TileContext(nc) as tc:
        with (
            tc.tile_pool(name="const", bufs=1) as cp,
            tc.tile_pool(name="big", bufs=1) as bp,
            tc.tile_pool(name="one", bufs=1) as op,
            tc.tile_pool(name="ta", bufs=2) as ta,    # ndt / u1c / u2c
            tc.tile_pool(name="tb", bufs=2) as tb,    # ug1c / ug2c / fp8 staging
            tc.tile_pool(name="sm", bufs=2) as sp,
            tc.tile_pool(name="pnd", bufs=2, space="PSUM") as pnd,
            tc.tile_pool(name="pcv", bufs=2, space="PSUM") as pcv,
            tc.tile_pool(name="pst", bufs=2, space="PSUM") as pst,
        ):
            # ---- constants (shared by both samples) ----
            w1a = cp.tile([P, 3, 512], f16)
            nc.sync.dma_start(w1a, wview("w1a").bitcast(f16).rearrange(
                "(ko p m) -> p ko m", p=P, m=512))
            w1d = cp.tile([P, 3, 512], f16)
            nc.sync.dma_start(w1d, wview("w1d").bitcast(f16).rearrange(
                "(ko p m) -> p ko m", p=P, m=512))
            w2a = cp.tile([P, 4, C], f16)
            nc.sync.dma_start(w2a, wview("w2a").bitcast(f16).rearrange(
                "(ko p m) -> p ko m", p=P, m=C))
            w2d = cp.tile([P, 4, C], f16)
            nc.sync.dma_start(w2d, wview("w2d").bitcast(f16).rearrange(
                "(ko p m) -> p ko m", p=P, m=C))
            g1t = cp.tile([P, 4], f32)
            nc.sync.dma_start(g1t, wview("g1t").rearrange("(p a) -> p a", p=P))
            b1t = cp.tile([P, 4], f32)
            nc.sync.dma_start(b1t, wview("b1t").rearrange("(p a) -> p a", p=P))
            g2t = cp.tile([P, 3], f32)
            nc.sync.dma_start(g2t, wview("g2t").rearrange("(p a) -> p a", p=P))
            b2t = cp.tile([P, 3], f32)
            nc.sync.dma_start(b2t, wview("b2t").rearrange("(p a) -> p a", p=P))
            sel1 = cp.tile([P, 4, 4], f32)
            nc.sync.dma_start(sel1, wview("sel1").rearrange(
                "(p a b) -> p a b", p=P, a=4))
            sel1t = cp.tile([4, 4, P], f32)
            nc.sync.dma_start(sel1t, wview("sel1t").rearrange(
                "(p a b) -> p a b", p=4, a=4))
            sel2 = cp.tile([P, 3, 4], f32)
            nc.sync.dma_start(sel2, wview("sel2").rearrange(
                "(p a b) -> p a b", p=P, a=3))
            sel2t = cp.tile([4, 3, P], f32)
            nc.sync.dma_start(sel2t, wview("sel2t").rearrange(
                "(p a b) -> p a b", p=4, a=3))
            epst = cp.tile([4, 1], f32)
            nc.vector.memset(epst, EPS)
            zt = cp.tile([P, 1], f32)
            nc.vector.memset(zt, 0.0)

            def knn_stage(nkeys, lq_t, rk_t):
                """Exact fp32 kNN scores + top-4.  Returns wl4 [P, 256] i16."""
                nch = nkeys // 512
                idx8 = sp.tile([P, 8, NT], dt.uint16, tag="idx8")
                for t in range(NT):
                    ndt = ta.tile([P, nkeys], f32, tag="ta")
                    for ch in range(nch):
                        ps = pnd.tile([P, 512], f32, tag="pnd")
                        nc.tensor.matmul(ps, lq_t[:, t * P:(t + 1) * P],
                                         rk_t[:, ch * 512:(ch + 1) * 512],
                                         start=True, stop=True)
                        nc.scalar.copy(ndt[:, ch * 512:(ch + 1) * 512], ps)
                    mx8 = sp.tile([P, 8], f32, tag="mx8")
                    nc.vector.max(out=mx8, in_=ndt)
                    nc.vector.max_index(out=idx8[:, :, t], in_max=mx8,
                                        in_values=ndt)
                idx4 = sp.tile([P, 4, NT], i16, tag="idx4")
                nc.vector.tensor_copy(idx4, idx8[:, 0:4, :].bitcast(i16))
                # wrapped gather list for ap_gather (i = j*1024 + q)
                wl4 = sp.tile([P, 4, 8, 8], i16, tag="wl4")  # [p, j, t, a]
                for a in range(8):
                    nc.sync.dma_start(
                        wl4[0:16, :, :, a], idx4[16 * a:16 * (a + 1)])
                wl4f = wl4.rearrange("p j t a -> p (j t a)")
                for g in range(1, 8):
                    nc.sync.dma_start(wl4f[16 * g:16 * (g + 1), :], wl4f[0:16, :])
                return wl4f

            def gn_prelu(n_c, maxed, sy, ssq, sel, selt, gt, bt, n_grp, out_t):
                """GroupNorm from raw per-partition sums + Prelu on maxed."""
                st2 = sp.tile([P, n_c, 2], f32, tag="st2")
                nc.vector.tensor_copy(st2[:, :, 0], sy)
                nc.vector.tensor_copy(st2[:, :, 1], ssq)
                psg = pst.tile([4, 2], f32, tag="psg")
                for c in range(n_c):
                    nc.tensor.matmul(psg, sel[:, c, :], st2[:, c, :],
                                     start=(c == 0), stop=(c == n_c - 1))
                gv = sp.tile([4, 2], f32, tag="gv")
                nc.scalar.mul(gv, psg, 1.0 / n_grp)
                msq = sp.tile([4, 1], f32, tag="msq")
                nc.vector.tensor_mul(msq, gv[:, 0:1], gv[:, 0:1])
                varg = sp.tile([4, 1], f32, tag="varg")
                nc.vector.tensor_sub(varg, gv[:, 1:2], msq)
                sd = sp.tile([4, 1], f32, tag="sd")
                nc.scalar.activation(sd, varg, AF.Sqrt, bias=epst[:], scale=1.0)
                mbv = sp.tile([4, 2], f32, tag="mbv")
                nc.vector.reciprocal(mbv[:, 1:2], sd)
                nc.vector.tensor_copy(mbv[:, 0:1], gv[:, 0:1])
                mv = sp.tile([P, n_c, 2], f32, tag="mv")
                for c in range(n_c):
                    psb = pst.tile([P, 2], f32, tag="psb")
                    nc.tensor.matmul(psb, selt[:, c, :], mbv, start=True, stop=True)
                    nc.scalar.copy(mv[:, c, :], psb)
                sv = sp.tile([P, n_c], f32, tag="sv")
                bv = sp.tile([P, n_c], f32, tag="bv")
                tmp = sp.tile([P, n_c], f32, tag="gtmp")
                nc.vector.tensor_mul(sv, gt, mv[:, :, 1])
                nc.vector.tensor_mul(tmp, mv[:, :, 0], sv)
                nc.vector.tensor_sub(bv, bt, tmp)
                for c in range(n_c):
                    nc.scalar.activation(
                        out_t[:, c, :], maxed[:, c, :], AF.Prelu,
                        bias=bv[:, c:c + 1], scale=sv[:, c:c + 1], alpha=ALPHA)

            def conv_plane(w, src, n_ko, m, out_c):
                """out_c[P, n] f32 <- sum_ko w[:, ko, m*P:(m+1)*P].T @ src[:, ko, :]"""
                n = src.shape[2]
                for ch in range(n // 512):
                    ps = pcv.tile([P, 512], f32, tag="pcv")
                    for ko in range(n_ko):
                        nc.tensor.matmul(ps, w[:, ko, m * P:(m + 1) * P],
                                         src[:, ko, ch * 512:(ch + 1) * 512],
                                         start=(ko == 0), stop=(ko == n_ko - 1))
                    nc.scalar.copy(out_c[:, ch * 512:(ch + 1) * 512], ps)

            def block(n_c, n_ko, wa, wd, src_u, src_v, wl4, nelems, sy, ssq, maxed):
                """Per-plane: conv U, gather, +V, stats, maxj.  V computed first."""
                vt = op.tile([P, n_c, GD], f16, tag="v")
                for m in range(n_c):
                    for ch in range(GD // 512):
                        ps = pcv.tile([P, 512], f32, tag="pcv")
                        for ko in range(n_ko):
                            nc.tensor.matmul(ps, wd[:, ko, m * P:(m + 1) * P],
                                             src_v[:, ko, ch * 512:(ch + 1) * 512],
                                             start=(ko == 0), stop=(ko == n_ko - 1))
                        nc.scalar.copy(vt[:, m, ch * 512:(ch + 1) * 512], ps)
                for c in range(n_c):
                    uc = ta.tile([P, nelems], f32, tag="ta")
                    conv_plane(wa, src_u, n_ko, c, uc)
                    ugc = tb.tile([P, 4 * GD], f32, tag="tb")
                    nc.gpsimd.ap_gather(
                        out_ap=ugc[:], in_ap=uc[:], idxs_ap=wl4,
                        channels=P, num_elems=nelems, d=1, num_idxs=4 * GD)
                    # y = ug + v (j-major), with sum accumulation
                    yc = sp.tile([P, 4, GD], f16, tag="yc")
                    nc.vector.scalar_tensor_tensor(
                        out=yc, in0=ugc.rearrange("p (j q) -> p j q", j=4),
                        scalar=0.0, in1=vt[:, c:c + 1, :].to_broadcast([P, 4, GD]),
                        op0=ALU.add, op1=ALU.add, accum_out=sy[:, c:c + 1])
                    # sum of squares via in-place ACT square
                    nc.scalar.activation(yc, yc, AF.Square, bias=zt[:], scale=1.0,
                                         accum_out=ssq[:, c:c + 1])
                    # max over j on ungathered-plus-v: max_j(ug) + v
                    ugr = ugc.rearrange("p (j q) -> p j q", j=4)
                    m0 = sp.tile([P, GD], f16, tag="m0")
                    m1 = sp.tile([P, GD], f16, tag="m1")
                    nc.vector.tensor_max(m0, ugr[:, 0, :], ugr[:, 1, :])
                    nc.vector.tensor_max(m1, ugr[:, 2, :], ugr[:, 3, :])
                    nc.vector.tensor_max(m0, m0, m1)
                    nc.vector.tensor_add(maxed[:, c, :], m0, vt[:, c, :])
                return vt

            for s in range(BC):
                # ---- per-sample loads ----
                lqo, _ = _SEC["lq"]
                lqt = op.tile([4, GD], f32, tag="lqt")
                nc.sync.dma_start(lqt, blob_d[0, lqo + s * 4 * GD:
                                              lqo + (s + 1) * 4 * GD]
                                  .rearrange("(r g) -> r g", r=4))
                r1o, _ = _SEC["rk1"]
                rk1t = op.tile([4, GS], f32, tag="rk1t")
                nc.sync.dma_start(rk1t, blob_d[0, r1o + s * 4 * GS:
                                               r1o + (s + 1) * 4 * GS]
                                  .rearrange("(r g) -> r g", r=4))
                r2o, _ = _SEC["rk2"]
                rk2t = op.tile([4, GD], f32, tag="rk2t")
                nc.sync.dma_start(rk2t, blob_d[0, r2o + s * 4 * GD:
                                               r2o + (s + 1) * 4 * GD]
                                  .rearrange("(r g) -> r g", r=4))
                fso, _ = _SEC["fs"]
                fs8 = tb.tile([P, 3, GS], i8, tag="tb")
                nc.sync.dma_start(
                    fs8, blob_d[0, fso + s * C * GS // 4:
                                fso + (s + 1) * C * GS // 4]
                    .bitcast(i8).rearrange("(ko p g) -> p ko g", p=P, g=GS))
                ssco, _ = _SEC["fssc"]
                fssc = sp.tile([P, 3], f32, tag="fssc")
                nc.sync.dma_start(fssc, blob_d[0, ssco + s * C:ssco + (s + 1) * C]
                                  .rearrange("(ko p) -> p ko", p=P))
                fs = bp.tile([P, 3, GS], f16, tag="fs_h")
                nc.vector.tensor_copy(fs, fs8)
                nc.vector.tensor_tensor(
                    out=fs, in0=fs, in1=fssc[:, :, None].to_broadcast([P, 3, GS]),
                    op=ALU.mult)
                fqo, _ = _SEC["fq"]
                fq8 = sp.tile([P, 3, GD], i8, tag="fq8")
                nc.sync.dma_start(
                    fq8, blob_d[0, fqo + s * C * GD // 4:
                                fqo + (s + 1) * C * GD // 4]
                    .bitcast(i8).rearrange("(ko p g) -> p ko g", p=P, g=GD))
                qsco, _ = _SEC["fqsc"]
                fqsc = sp.tile([P, 3], f32, tag="fqsc")
                nc.sync.dma_start(fqsc, blob_d[0, qsco + s * C:qsco + (s + 1) * C]
                                  .rearrange("(ko p) -> p ko", p=P))
                fq = op.tile([P, 3, GD], f16, tag="fq")
                nc.vector.tensor_copy(fq, fq8)
                nc.vector.tensor_tensor(
                    out=fq, in0=fq, in1=fqsc[:, :, None].to_broadcast([P, 3, GD]),
                    op=ALU.mult)

                # ---- kNN stage 1 & 2 ----
                wl4_1 = knn_stage(GS, lqt, rk1t)
                wl4_2 = knn_stage(GD, lqt, rk2t)

                # ---- block 1 ----
                sy1 = op.tile([P, 4], f32, tag="sy1")
                ssq1 = op.tile([P, 4], f32, tag="ssq1")
                maxed1 = op.tile([P, 4, GD], f16, tag="maxed")
                block(4, 3, w1a, w1d, fs, fq, wl4_1, GS, sy1, ssq1, maxed1)
                h = op.tile([P, 4, GD], f16, tag="fs_h")
                gn_prelu(4, maxed1, sy1, ssq1, sel1, sel1t, g1t, b1t,
                         P * 4 * GD, h)

                # ---- block 2 ----
                sy2 = op.tile([P, 3], f32, tag="sy2")
                ssq2 = op.tile([P, 3], f32, tag="ssq2")
                maxed2 = op.tile([P, 3, GD], f16, tag="maxed")
                block(3, 4, w2a, w2d, h, h, wl4_2, GD, sy2, ssq2, maxed2)
                outp = op.tile([P, 3, GD], f16, tag="outp")
                gn_prelu(3, maxed2, sy2, ssq2, sel2, sel2t, g2t, b2t,
                         96 * 4 * GD, outp)
                # int8 output quantization: per-(p, cc) amax scale.  DVE
                # float->int8 conversion is RNE with saturation (verified
                # on HW), so the quantization error is sc/sqrt(12).
                sct = sp.tile([P, 3], f32, tag="sct")
                for cc in range(3):
                    ab = sp.tile([P, GD], f16, tag="ab")
                    nc.scalar.activation(ab, outp[:, cc, :], AF.Abs,
                                         bias=zt[:], scale=1.0)
                    mx8o = sp.tile([P, 8], f32, tag="mx8o")
                    nc.vector.max(out=mx8o, in_=ab)
                    nc.vector.tensor_copy(sct[:, cc:cc + 1], mx8o[:, 0:1])
                nc.vector.tensor_scalar(out=sct, in0=sct, scalar1=1e-20,
                                        scalar2=None, op0=ALU.max)
                rec = sp.tile([P, 3], f32, tag="rec")
                nc.vector.reciprocal(rec, sct)
                rsc = sp.tile([P, 3], f32, tag="rsc")
                nc.vector.tensor_scalar(out=rsc, in0=rec, scalar1=127.0,
                                        scalar2=None, op0=ALU.mult)
                qo = op.tile([P, 3, GD], i8, tag="qo")
                for cc in range(3):
                    qf = sp.tile([P, GD], f16, tag="qf")
                    nc.scalar.activation(qf, outp[:, cc, :], AF.Prelu,
                                         bias=zt[:], scale=rsc[:, cc:cc + 1],
                                         alpha=1.0)
                    nc.vector.tensor_copy(qo[:, cc, :], qf)
                sco = sp.tile([P, 3], f32, tag="sco")
                nc.vector.tensor_scalar(out=sco, in0=sct, scalar1=1.0 / 127.0,
                                        scalar2=None, op0=ALU.mult)
                base = s * SWRD
                nc.sync.dma_start(
                    out_d[0, base:base + QW].bitcast(i8)
                    .rearrange("(p c g) -> p c g", p=P, c=3), qo)
                nc.sync.dma_start(
                    out_d[0, base + QW:base + QW + P * 3]
                    .rearrange("(p c) -> p c", p=P), sco)

    nc.compile()
    return nc


# ---------------- host runner ----------------
_STATE = None


class _State:
    pass


def _get_state():
    global _STATE
    if _STATE is not None:
        return _STATE
    import jax
    from jax.sharding import Mesh, PartitionSpec, NamedSharding
    from jax.experimental.shard_map import shard_map

    st = _State()
    st.nc = _build()
    nc = st.nc
    bass2jax.install_neuronx_cc_hook()
    partition_name = (nc.partition_id_tensor.name
                      if nc.partition_id_tensor else None)
    in_names = ["blob", "wblob", "out"]
    if partition_name:
        in_names.append(partition_name)
    out_avals = (jax.core.ShapedArray((1, OW), np.float32),)

    def _body(blob, wblob, outbuf):
        # `outbuf` is a dead parameter: the neuronx_cc_hook renames the BIR
        # "out" tensor to output0 (bound to the custom-call RESULT buffer),
        # so this operand's content is never read.  The kernel writes every
        # element of the result, so no pre-zeroing is needed either.
        operands = [blob, wblob, outbuf]
        if partition_name:
            operands.append(bass2jax.partition_id_tensor())
        return bass2jax._bass_exec_p.bind(
            *operands, out_avals=out_avals, in_names=tuple(in_names),
            out_names=("out",), lowering_input_output_aliases=(),
            sim_require_finite=True, sim_require_nnan=True, nc=nc)[0]

    devices = jax.devices()[:NCORE]
    mesh = Mesh(np.asarray(devices), ("core",))
    st.sharding = NamedSharding(mesh, PartitionSpec("core"))
    st.jitted = jax.jit(
        shard_map(_body, mesh=mesh,
                  in_specs=(PartitionSpec("core"),) * 3,
                  out_specs=PartitionSpec("core"), check_rep=False))
    st.dead_out = jax.device_put(np.zeros((NCORE, OW), np.float32),
                                 st.sharding)
    st.blob = np.zeros((NCORE, NW), np.float32)
    st.wblob = np.zeros((NCORE, WW), np.float32)
    st.wkey = None       # byte snapshot of (W1, W2, g1, b1, g2, b2)
    st.wdev = None       # device-resident weight blob
    # constant sections: selector matrices
    sel1 = np.zeros((P, 4, 4), np.float32)
    for c in range(4):
        for p in range(P):
            sel1[p, c, (c * P + p) // 128] = 1.0
    sel2 = np.zeros((P, 3, 4), np.float32)
    for c in range(3):
        for p in range(P):
            sel2[p, c, (c * P + p) // 96] = 1.0
    _fill(st.wblob, _WSEC, "sel1", sel1)
    _fill(st.wblob, _WSEC, "sel1t", np.ascontiguousarray(sel1.transpose(2, 1, 0)))
    _fill(st.wblob, _WSEC, "sel2", sel2)
    _fill(st.wblob, _WSEC, "sel2t", np.ascontiguousarray(sel2.transpose(2, 1, 0)))
    _STATE = st
    return st


def _fill(blob, secs, name, arr):
    o, w = secs[name]
    blob[:, o:o + w] = arr.reshape(1, -1).astype(np.float32)


def _view(blob, secs, name, shape, dtype=np.float32):
    """Per-core view of a blob section, reshaped to (NCORE, *shape)."""
    o, w = secs[name]
    v = blob[:, o:o + w]
    if dtype != np.float32:
        v = v.view(dtype)[:, :int(np.prod(shape))]
    return v.reshape(NCORE, *shape)


def _weights_changed(st, parts):
    key = b"".join(np.ascontiguousarray(p).tobytes() for p in parts)
    if st.wkey is not None and st.wkey == key:
        return False
    st.wkey = key
    return True


_POOL = None
_TMP = {}


def _tmp(key, shape):
    t = _TMP.get(key)
    if t is None:
        t = np.empty(shape, np.float32)
        _TMP[key] = t
    return t


def _quant_i8(x, q_out, sc_out):
    """Per-(core, sample, channel) symmetric int8 quantization, threaded
    over cores.

    sc_out layout per sample is (ko p): channel c = ko*128 + p maps to
    element ko*128 + p, matching the device's [p, ko] scale tile load.
    """
    global _POOL
    if _POOL is None:
        from concurrent.futures import ThreadPoolExecutor
        _POOL = ThreadPoolExecutor(NCORE)

    def one(c):
        xc = x[c]                                      # [BC, C, G]
        t = _tmp((xc.shape, c), xc.shape)
        amax = np.maximum(xc.max(axis=-1), -xc.min(axis=-1))   # [BC, C]
        rsc = 127.0 / np.maximum(amax, 1e-30)
        np.multiply(xc, rsc[..., None], out=t)
        np.rint(t, out=t)
        np.copyto(q_out[c], t, casting="unsafe")
        sc_out[c] = (1.0 / rsc).reshape(sc_out.shape[1:])

    list(_POOL.map(one, range(NCORE)))


def kernel(**inputs):
    import jax
    st = _get_state()
    blob = st.blob

    inputs = {k: np.asarray(v) for k, v in inputs.items()}
    f = np.ascontiguousarray(inputs["f"], dtype=np.float32)
    f_q = np.ascontiguousarray(inputs["f_q"], dtype=np.float32)
    coor = np.ascontiguousarray(inputs["coor"], dtype=np.float32)
    coor_q = np.ascontiguousarray(inputs["coor_q"], dtype=np.float32)

    # big int8 payloads with per-(sample, channel) scales
    _quant_i8(f.reshape(NCORE, BC, C, GS),
              _view(blob, _SEC, "fs", (BC, C, GS), np.int8),
              _view(blob, _SEC, "fssc", (BC, C)))
    _quant_i8(f_q.reshape(NCORE, BC, C, GD),
              _view(blob, _SEC, "fq", (BC, C, GD), np.int8),
              _view(blob, _SEC, "fqsc", (BC, C)))

    # kNN rows (exact f32)
    k2s = (coor * coor).sum(axis=1)        # [16, GS] fp32, same order as ref
    k2q = (coor_q * coor_q).sum(axis=1)    # [16, GD]
    lq = _view(blob, _SEC, "lq", (BC, 4, GD))
    lq[:, :, 0:3, :] = coor_q.reshape(NCORE, BC, 3, GD)
    lq[:, :, 3, :] = 1.0
    rk1 = _view(blob, _SEC, "rk1", (BC, 4, GS))
    rk1[:, :, 0:3, :] = 2.0 * coor.reshape(NCORE, BC, 3, GS)
    rk1[:, :, 3, :] = -k2s.reshape(NCORE, BC, GS)
    rk2 = _view(blob, _SEC, "rk2", (BC, 4, GD))
    rk2[:, :, 0:3, :] = 2.0 * coor_q.reshape(NCORE, BC, 3, GD)
    rk2[:, :, 3, :] = -k2q.reshape(NCORE, BC, GD)

    dev = jax.device_put(blob, st.sharding)   # async; overlaps weight check

    # weights (device-resident unless changed)
    W1 = inputs["W1"].astype(np.float32, copy=False)
    W2 = inputs["W2"].astype(np.float32, copy=False)
    g1 = inputs["g1"].astype(np.float32, copy=False)
    b1 = inputs["b1"].astype(np.float32, copy=False)
    g2 = inputs["g2"].astype(np.float32, copy=False)
    b2 = inputs["b2"].astype(np.float32, copy=False)
    if st.wdev is None or _weights_changed(st, (W1, W2, g1, b1, g2, b2)):
        W1a, W1b = W1[:, :C], W1[:, C:]
        W2a, W2b = W2[:, :512], W2[:, 512:]
        wb = st.wblob
        np.copyto(_view(wb, _WSEC, "w1a", (C, 512), np.float16), W1a.T[None])
        np.copyto(_view(wb, _WSEC, "w1d", (C, 512), np.float16),
                  (W1b - W1a).T[None])
        np.copyto(_view(wb, _WSEC, "w2a", (512, C), np.float16), W2a.T[None])
        np.copyto(_view(wb, _WSEC, "w2d", (512, C), np.float16),
                  (W2b - W2a).T[None])
        _fill(wb, _WSEC, "g1t", np.ascontiguousarray(g1.reshape(4, P).T))
        _fill(wb, _WSEC, "b1t", np.ascontiguousarray(b1.reshape(4, P).T))
        _fill(wb, _WSEC, "g2t", np.ascontiguousarray(g2.reshape(3, P).T))
        _fill(wb, _WSEC, "b2t", np.ascontiguousarray(b2.reshape(3, P).T))
        st.wdev = jax.device_put(wb, st.sharding)

    out = st.jitted(dev, st.wdev, st.dead_out)
    res = np.asarray(out)                     # [8, OW] f32 container
    res3 = res.reshape(NCORE, BC, SWRD)
    q = res3[:, :, :QW].view(np.int8).reshape(NCORE, BC, P, 3, GD)
    sc = res3[:, :, QW:QW + P * 3].reshape(NCORE, BC, P, 3)
    y = q.astype(np.float32)
    y *= sc[..., None]
    return np.ascontiguousarray(y.transpose(0, 1, 3, 2, 4)).reshape(B, C, GD)
